# revision 34
# baseline (speedup 1.0000x reference)
"""DeepAR (2-layer LSTM, HID=128) forward on 8 Trainium2 NeuronCores.

Sharding: pure data parallelism. Batch 2048 -> 256 rows per core; LSTM
weights / embedding products replicated; no cross-device communication.

Device kernel layout ("gates on partitions", software-pipelined):
  - The per-core batch of 256 runs as TWO independent 128-row recurrence
    chains.  The recurrence is a ROTATED software pipeline: phase p
    executes layer-1 of step p, the sigmoid of layer-2 of step p-1, and
    the cell update/tanh/h2 of layer-2 of step p-2.  Every ACT-engine
    instruction's inputs are therefore produced at least one phase (or
    early in the same phase) before use, which keeps the in-order ACT
    queue [sig1 x2, tanh2, tanh1 x2, sig2 x2] nearly stall-free -- the
    ACT engine (sigmoid/tanh, ~3.4 us/phase busy) is the bottleneck at
    ~85% occupancy.
  - State tiles are [dim=128 partitions, batch free].  Gate
    pre-activations accumulate in PSUM as one [128, 1024] tile (2 banks)
    per layer-phase, both chains side by side; per-chain sigmoids read
    their [128, 512] half (per-chain ACT granularity beats merged wide
    instructions: the scheduler fills gaps with the other chain's work).
    tanh of layer 2 runs merged across chains ([128,256]) -- it is off
    the recurrence-critical path, so only its ~185ns fixed ACT overhead
    matters.
  - L1's bias rides a constant ones-row appended to the input (K=66).
    L2's bias is added with one K=4 "indicator" matmul per chain.
  - g-gate rows of every weight/bias are pre-scaled by 2 on the host so a
    single Sigmoid activation covers all four gates (tanh(g) = 2*sig(2g)-1,
    fixed up on the vector engine).
  - h2 history lives in an 8-step SBUF ring (readers lag writes by <= 2
    steps); the output projection packs three [14,512] psum chunks per
    bank at partition bases 0/32/64 so ONE [78,512] staging copy feeds
    the pack DMAs (DVE copy cost depends on free size only).  Epilogue:
    ln(1+exp) softplus + denormalization, unpacked on the host.

kernel(**inputs) is self-contained: hardcodes shapes, shards, compiles via
bass/Tile, runs on cores 0-7 through bass_utils.run_bass_kernel_spmd, and
reassembles the full [2048, 192, 14] float32 output.
"""

import math

import numpy as np
import ml_dtypes

import concourse.bass as bass
import concourse.mybir as mybir
from concourse.tile import TileContext
from concourse import bacc, bass_utils

F32 = mybir.dt.float32
BF16 = mybir.dt.bfloat16
AF = mybir.ActivationFunctionType
ALU = mybir.AluOpType

# Model dims (hardcoded from the problem spec)
B = 2048
SEQ = 168
PRED = 24
W = SEQ + PRED          # 192
TGT = 7
TNUM = 4
TCAT = 3
MNUM = 2
MCAT = 2
COV = 4
HID = 128
INP = 65                # 7 + 28 + 4 + 26
INPX = INP + 1          # + ones row for the L1 bias
NCORES = 8
BS = B // NCORES        # 256 batch rows per core
NG = 4 * BS             # 1024: four gate chunks along psum free dim
XCH = 1                 # timesteps per input-chunk DMA
NPROJ = W * BS // 512   # 96 projection chunks of [14, 512]
PGRP = 16               # chunks per partition-group in the packed output
PCC = NPROJ // PGRP     # 6 free-dim column groups

_CACHE = {}


# --------------------------------------------------------------------------
# host-side preprocessing
# --------------------------------------------------------------------------

def _host_prep(inputs):
    ge = inputs["given_enc"].astype(np.float32)
    xe = inputs["x_enc"].astype(np.float32)
    xm = inputs["x_mark_enc"].astype(np.float32)
    mx = inputs["meta_x"].astype(np.float32)
    tembs = [inputs["temb0"], inputs["temb1"], inputs["temb2"]]
    membs = [inputs["memb0"], inputs["memb1"]]

    # categorical embedding gathers
    ge_cat = [tembs[i][ge[:, :, TNUM + i].astype(np.int32)] for i in range(TCAT)]
    mx_cat = [membs[i][mx[:, MNUM + i].astype(np.int32)] for i in range(MCAT)]

    # instance norm over the time axis
    norm_mean = xe.mean(axis=1, keepdims=True)                 # [B,1,7]
    xc = xe - norm_mean
    norm_std = np.sqrt((xc * xc).mean(axis=1, keepdims=True) + 1e-5)
    xn = xc / norm_std

    # teacher forcing shift
    idx = np.clip(np.arange(W) - 1, 0, SEQ - 1)
    prev_y = xn[:, idx, :]                                     # [B,W,7]

    mx_embed = np.concatenate([mx[:, :MNUM]] + mx_cat, axis=-1)   # [B,26]
    mx_b = np.broadcast_to(mx_embed[:, None, :], (B, W, mx_embed.shape[-1]))
    inp = np.concatenate(
        [prev_y, ge[:, :, :TNUM]] + ge_cat + [xm, mx_b], axis=-1
    )                                                          # [B,W,65]
    return inp, norm_mean[:, 0, :], norm_std[:, 0, :]          # means/stds [B,7]


def _gscale(wT):
    """Scale the g-gate block (rows 2*HID:3*HID of the gate dim) by 2.
    wT is [K, 4*HID] (gate dim along columns)."""
    w = wT.copy()
    w[:, 2 * HID:3 * HID] *= 2.0
    return w


def _host_weights(inputs):
    bf = ml_dtypes.bfloat16
    w = {}
    # L1: input weights + combined bias as an extra contraction row
    wih0T = np.concatenate(
        [inputs["Wih0"].T, (inputs["bih0"] + inputs["bhh0"])[None, :]], axis=0
    )                                                          # [66, 512]
    w["wih0"] = _gscale(wih0T).astype(bf)
    w["whh0"] = _gscale(inputs["Whh0"].T).astype(bf)           # [128, 512]
    w["wih1"] = _gscale(inputs["Wih1"].T).astype(bf)           # [128, 512]
    w["whh1"] = _gscale(inputs["Whh1"].T).astype(bf)           # [128, 512]
    b2 = _gscale((inputs["bih1"] + inputs["bhh1"])[None, :])[0]  # [512]
    w["b2all"] = b2.reshape(4, HID).astype(bf)                 # [4,128]
    ind = np.zeros((4, 512), np.float32)
    for k in range(4):
        ind[k, 128 * k:128 * (k + 1)] = 1.0
    w["ind"] = ind.astype(bf)
    w["wms"] = np.concatenate([inputs["Wm"], inputs["Ws"]], axis=0).T.astype(bf)  # [128,14]
    return w


# --------------------------------------------------------------------------
# device kernel builder
# --------------------------------------------------------------------------

def build_module(nsteps=W, MODES=('pc', 'l2m'), ORDER='R'):
    # Bacc (not raw Bass): its compile() runs move_matmul_waits_to_ldweights
    # and generate_event_semaphores, which walrus needs (max 1 wait/inst).
    nc = bacc.Bacc("TRN2", target_bir_lowering=False, debug=False,
                   enable_asserts=False, num_devices=NCORES)
    nproj = nsteps * BS // 512
    pcc = max(1, nproj // PGRP)
    ncols = nsteps * BS

    inp_d = nc.dram_tensor("inp", [INPX, ncols], BF16, kind="ExternalInput").ap()
    wih0_d = nc.dram_tensor("wih0", [INPX, 4 * HID], BF16, kind="ExternalInput").ap()
    whh0_d = nc.dram_tensor("whh0", [HID, 4 * HID], BF16, kind="ExternalInput").ap()
    wih1_d = nc.dram_tensor("wih1", [HID, 4 * HID], BF16, kind="ExternalInput").ap()
    whh1_d = nc.dram_tensor("whh1", [HID, 4 * HID], BF16, kind="ExternalInput").ap()
    b2all_d = nc.dram_tensor("b2all", [4, HID], BF16, kind="ExternalInput").ap()
    ind_d = nc.dram_tensor("ind", [4, 512], BF16, kind="ExternalInput").ap()
    wms_d = nc.dram_tensor("wms", [HID, 2 * TGT], BF16, kind="ExternalInput").ap()
    stdp_d = nc.dram_tensor("stdp", [TGT * PGRP, BS], F32, kind="ExternalInput").ap()
    meanp_d = nc.dram_tensor("meanp", [TGT * PGRP, BS], F32, kind="ExternalInput").ap()
    bsp_d = nc.dram_tensor("bsp", [TGT * PGRP, 1], F32, kind="ExternalInput").ap()

    means_d = nc.dram_tensor("means", [TGT * PGRP, 512 * pcc], F32,
                             kind="ExternalOutput").ap()
    sigmas_d = nc.dram_tensor("sigmas", [TGT * PGRP, 512 * pcc], F32,
                              kind="ExternalOutput").ap()

    with TileContext(nc) as tc:
        with tc.tile_pool(name="singles", bufs=1) as singles, \
             tc.tile_pool(name="xin", bufs=3) as xpool, \
             tc.tile_pool(name="vec", bufs=2) as vp, \
             tc.tile_pool(name="sig", bufs=2) as sigp, \
             tc.tile_pool(name="h1p", bufs=2) as h1p:

            def load(name, dram, shape, dtype=BF16):
                t = singles.tile(shape, dtype, tag=name)
                nc.sync.dma_start(out=t[:], in_=dram)
                return t

            from concourse.masks import make_identity
            ident = singles.tile([HID, HID], BF16, tag="ident")
            make_identity(nc, ident[:])
            wih0 = load("wih0", wih0_d, [INPX, 4 * HID])
            whh0 = load("whh0", whh0_d, [HID, 4 * HID])
            wih1 = load("wih1", wih1_d, [HID, 4 * HID])
            whh1 = load("whh1", whh1_d, [HID, 4 * HID])
            b2all = load("b2all", b2all_d, [4, HID])
            ind = load("ind", ind_d, [4, 512])
            wms = load("wms", wms_d, [HID, 2 * TGT])
            stdp = load("stdp", stdp_d, [TGT * PGRP, BS], F32)
            meanp = load("meanp", meanp_d, [TGT * PGRP, BS], F32)
            bsp = load("bsp", bsp_d, [TGT * PGRP, 1], F32)

            RS = 8  # h2 history ring (steps); readers lag writes by <= 2 steps
            h2_hist = singles.tile([HID, RS * BS], BF16, tag="h2_ring")
            means_sb = singles.tile([TGT * PGRP, 512 * pcc], F32, tag="means_sb")
            sigraw_sb = singles.tile([TGT * PGRP, 512 * pcc], F32, tag="sigraw_sb")

            # Software-pipelined phase loop: phase p runs layer-1 of step p
            # AND layer-2 of step p-1.  Both halves depend only on phase
            # p-1 outputs (h1(p-1), c1(p-1), c2(p-2), h2(p-2)), never on
            # each other, so the in-order ACT queue
            #   [sig1, sig2, tanh1, tanh2]
            # never stalls on an intra-phase dependency chain: while L1's
            # sigmoid->cell->tanh chain percolates, the (independent) L2
            # work keeps the activation engine fed.  The two 128-row batch
            # chains share one [128,1024] gate psum tile (2 banks) per
            # layer-phase; MERGE_L1/MERGE_L2 pick whether the sigmoid/tanh
            # run as single wide ACT instructions (halving the ~185ns
            # fixed ACT overhead) or stay per-chain (shorter latency).
            CB = BS // 2   # 128 rows per chain
            NGc = 4 * CB   # 512: per-chain gate psum width (1 bank)
            st = dict(h1=None, c12=None, s2=None)
            x_tile = None

            import contextlib
            ctx = contextlib.ExitStack()
            gp = ctx.enter_context(
                tc.tile_pool(name="gp", bufs=3, space="PSUM"))
            projp = ctx.enter_context(
                tc.tile_pool(name="proj", bufs=2, space="PSUM"))
            stagep = ctx.enter_context(tc.tile_pool(name="stage", bufs=3))

            # SMODE: sigmoid instruction granularity
            #   'pc' = 4 per-chain [128,512]; 'm1'/'m2' = merge that layer's
            #   two chains into one [128,1024]; 'mm' = both layers merged.
            # TMODE: tanh granularity over the shared c12 tile [128,512]
            #   'pc' = 4x [128,128]; 'layer' = 2x [128,256] (cross-chain);
            #   'chain' = 2x [128,256] (cross-layer, per chain -- does NOT
            #   couple the two batch chains); 'all' = 1x [128,512].
            SMODE, TMODE = MODES

            def R(ap):
                # [128, 2*CB*k] -> [128, 2, CB*k]: chain-major free view
                return ap.rearrange("p (c q) -> p c q", c=2)

            def blk(l, ch):
                """c12/tt12 column block for (layer l in {0,1}, chain ch)."""
                if TMODE == 'chain':
                    return 2 * ch + l
                return 2 * l + ch

            def sig(g, prefix, merge):
                s = sigp.tile([HID, 2 * NGc], BF16, tag=f"s{prefix}")
                if merge:
                    nc.scalar.activation(s[:], g[:], AF.Sigmoid)
                else:
                    for ch in (0, 1):
                        csl = slice(NGc * ch, NGc * (ch + 1))
                        nc.scalar.activation(s[:, csl], g[:, csl], AF.Sigmoid)
                return s

            def cell_dve(t, s, cprev, c12, l, eng=None, veng=None):
                """Per-chain cell update (gfix/u/v/c) writing this
                layer's two column blocks of the shared c12 tile.  All
                operands live in SBUF, so the ops can run on either the
                vector engine or the (otherwise idle) GPSIMD engine --
                layer 2's cell runs on GPSIMD so it cannot delay layer
                1's recurrence-critical cell ops in the DVE queue."""
                eng = eng or nc.vector
                veng = veng or eng
                sv = R(s[:])
                si, sf = sv[:, :, 0:CB], sv[:, :, CB:2 * CB]
                sg = sv[:, :, 2 * CB:3 * CB]
                for ch in (0, 1):
                    b = blk(l, ch)
                    h = slice(CB * b, CB * (b + 1))
                    sic, sfc, sgc = si[:, ch], sf[:, ch], sg[:, ch]
                    gt = vp.tile([HID, CB], BF16, tag=f"gt{l}{ch}")
                    eng.tensor_scalar(gt[:], sgc, 2.0, 1.0,
                                      ALU.mult, ALU.subtract)
                    if t == 0:
                        eng.tensor_mul(c12[:, h], sic, gt[:])
                    else:
                        u = vp.tile([HID, CB], BF16, tag=f"u{l}{ch}")
                        eng.tensor_mul(u[:], sic, gt[:])
                        v = vp.tile([HID, CB], BF16, tag=f"v{l}{ch}")
                        veng.tensor_mul(v[:], sfc, cprev[:, h])
                        veng.tensor_add(c12[:, h], u[:], v[:])

            def tanh_emit(c12, tt12, layers):
                """Emit tanh instruction(s) covering the blocks of the
                present layers, per TMODE."""
                blocks = sorted(blk(l, ch) for l in layers for ch in (0, 1))
                merge = TMODE in ('layer', 'chain', 'all') or (
                    TMODE == 'l2m' and 1 in layers)
                if TMODE == 'all' and len(blocks) == 4:
                    nc.scalar.activation(tt12[:], c12[:], AF.Tanh)
                    return
                if merge:
                    # pair consecutive present blocks where contiguous
                    i = 0
                    while i < len(blocks):
                        if (i + 1 < len(blocks)
                                and blocks[i + 1] == blocks[i] + 1):
                            h = slice(CB * blocks[i], CB * (blocks[i] + 2))
                            nc.scalar.activation(tt12[:, h], c12[:, h],
                                                 AF.Tanh)
                            i += 2
                        else:
                            h = slice(CB * blocks[i], CB * (blocks[i] + 1))
                            nc.scalar.activation(tt12[:, h], c12[:, h],
                                                 AF.Tanh)
                            i += 1
                    return
                for b in blocks:
                    h = slice(CB * b, CB * (b + 1))
                    nc.scalar.activation(tt12[:, h], c12[:, h], AF.Tanh)

            def hmul(dst_ap, s, tt12, l):
                """h = sig_o * tanh(c), per chain."""
                so = R(s[:])[:, :, 3 * CB:4 * CB]
                for ch in (0, 1):
                    b = blk(l, ch)
                    nc.vector.tensor_mul(
                        dst_ap[:, CB * ch:CB * (ch + 1)], so[:, ch],
                        tt12[:, CB * b:CB * (b + 1)])

            def cell2_rat(t, s, cprev, c12, tt12):
                """Layer-2 cell update with both chains merged per DVE op
                (chain dim via 2D free APs) and tanh(c) computed as the
                rational x*(27+x^2)/(27+9*x^2) on the vector engine --
                |c2| < 0.35 empirically, where the Pade error is <1e-5 --
                freeing the ACT engine of layer 2's tanh entirely."""
                sv = R(s[:])
                si, sf = sv[:, :, 0:CB], sv[:, :, CB:2 * CB]
                sg = sv[:, :, 2 * CB:3 * CB]
                csl = slice(2 * CB, 4 * CB)   # layer-2 blocks (layer-major)
                cv = c12[:, csl]
                gt = vp.tile([HID, BS], BF16, tag="gt2m")
                nc.vector.tensor_scalar(R(gt[:]), sg, 2.0, 1.0,
                                        ALU.mult, ALU.subtract)
                if t == 0:
                    nc.vector.tensor_mul(R(cv), si, R(gt[:]))
                else:
                    u = vp.tile([HID, BS], BF16, tag="u2m")
                    nc.vector.tensor_mul(R(u[:]), si, R(gt[:]))
                    v = vp.tile([HID, BS], BF16, tag="v2m")
                    nc.vector.tensor_mul(R(v[:]), sf, R(cprev[:, csl]))
                    nc.vector.tensor_add(cv, u[:], v[:])
                tsq = vp.tile([HID, BS], BF16, tag="tsq2")
                nc.vector.tensor_mul(tsq[:], cv, cv)
                nn = vp.tile([HID, BS], BF16, tag="nn2")
                nc.vector.scalar_tensor_tensor(nn[:], tsq[:], 27.0, cv,
                                               ALU.add, ALU.mult)
                dd = vp.tile([HID, BS], BF16, tag="dd2")
                nc.vector.tensor_scalar(dd[:], tsq[:], 9.0, 27.0,
                                        ALU.mult, ALU.add)
                nc.vector.tensor_tensor(tt12[:, csl], nn[:], dd[:],
                                        ALU.divide)

            def hmul2_m(dst_ap, s, tt12):
                """Merged-chain h2 = sig_o * tanh(c2)."""
                so = R(s[:])[:, :, 3 * CB:4 * CB]
                nc.vector.tensor_mul(R(dst_ap), so, R(tt12[:, 2 * CB:4 * CB]))

            # Rotated software pipeline.  Phase p emits:
            #   L1 of step p          (g1 MMs, sig1, cell1, tanh1, h1)
            #   sig2 of step p-1      (g2 MMs + sigmoid only)
            #   cell2..h2 of step p-2 (consuming the sigma2 emitted last
            #                          phase -- ready at phase START)
            # Every ACT instruction's inputs are thus produced >= one
            # phase earlier or early in this phase, so the in-order ACT
            # queue [sig1, tanh2, tanh1, sig2] runs back-to-back; the
            # recurrence-critical path (sig1 -> cell1 -> tanh1 -> h1 ->
            # next-phase h-MMs) has a full phase of slack.
            # NOTE: g2's h-part MMs read h2_hist written THIS phase, so
            # they are emitted after the h2-muls (tile dep tracking is
            # program-order based); bias+x-part go early, h-part late,
            # with the accumulation-group stop on the last h-MM.
            for p in range(nsteps + 2):
                has1 = p < nsteps           # layer-1 work for step p
                hasg2 = 1 <= p <= nsteps    # g2 MMs + sigma2 of step p-1
                hasc2 = 2 <= p              # cell2..h2 of step p-2
                t2 = p - 1                  # sigma2's step
                tc2 = p - 2                 # cell2's step

                def load_x(step):
                    t = xpool.tile([INPX, XCH * BS], BF16, tag="x")
                    nx = min(XCH, nsteps - step)
                    nc.sync.dma_start(
                        out=t[:, :nx * BS],
                        in_=inp_d[:, step * BS:(step + nx) * BS])
                    return t

                def emit_x_mms(g, step, x_t, last):
                    xo = (step % XCH) * BS
                    for ch in (0, 1):
                        xt = x_t[:, xo + CB * ch:xo + CB * (ch + 1)]
                        for m in range(4):
                            sl = slice(NGc * ch + CB * m,
                                       NGc * ch + CB * (m + 1))
                            nc.tensor.matmul(
                                g[:, sl], wih0[:, HID * m:HID * (m + 1)],
                                xt, start=True, stop=last)

                # ---- layer-1 matmuls.  The x-part MMs for step p+1 were
                # emitted LAST phase (they need only the prefetched x), so
                # the PE dispatches them during its mid-phase idle window
                # instead of serializing ~16 dispatches (~1.3us) between
                # h1(p)-ready and sigma1(p+1); here only the h-part lands.
                g1 = None
                if has1:
                    if p == 0:
                        x_next = load_x(0)
                        g1 = gp.tile([HID, 2 * NGc], F32, tag="g")
                        emit_x_mms(g1, 0, x_next, last=True)
                    else:
                        g1 = st["g1n"]
                        for ch in (0, 1):
                            for m in range(4):
                                sl = slice(NGc * ch + CB * m,
                                           NGc * ch + CB * (m + 1))
                                nc.tensor.matmul(
                                    g1[:, sl],
                                    whh0[:, HID * m:HID * (m + 1)],
                                    st["h1"][:, CB * ch:CB * (ch + 1)],
                                    start=False, stop=True)

                # ---- matmuls: layer 2 of step t2, bias + x-part only
                # (st["h1"] holds h1(t2) here; h-part emitted below)
                g2 = None
                if hasg2:
                    g2 = gp.tile([HID, 2 * NGc], F32, tag="g")
                    for ch in (0, 1):
                        csl = slice(NGc * ch, NGc * (ch + 1))
                        nc.tensor.matmul(g2[:, csl], b2all[:], ind[:],
                                         start=True, stop=False)
                        for m in range(4):
                            sl = slice(NGc * ch + CB * m,
                                       NGc * ch + CB * (m + 1))
                            nc.tensor.matmul(
                                g2[:, sl], wih1[:, HID * m:HID * (m + 1)],
                                st["h1"][:, CB * ch:CB * (ch + 1)],
                                start=False,
                                stop=(t2 == 0 and m == 3))

                c12 = vp.tile([HID, 2 * BS], BF16, tag="c12")
                tt12 = vp.tile([HID, 2 * BS], BF16, tag="tt12")

                # ---- sigma1(p), then cell2(p-2) [ready at phase start],
                # then cell1(p): ACT queue [s1,s1,t2,t2,t1,t1,s2,s2],
                # DVE queue [cell2, cell1, h2, h1].
                s1 = None
                if has1:
                    s1 = sig(g1, "1", SMODE in ('m1', 'mm'))
                if hasc2:
                    cell_dve(tc2, st["s2"], st["c12"], c12, 1)
                if has1:
                    cell_dve(p, s1, st["c12"], c12, 0)
                    tanh_emit(c12, tt12, [0])
                    h1 = h1p.tile([HID, BS], BF16, tag="h1")
                    hmul(h1[:], s1, tt12, 0)
                # h2-muls AFTER the (next-phase-critical) h1-muls in the
                # DVE queue; g2's h-part MMs after the h2 write (tile dep
                # tracking is program-order based)
                # tanh2 + h2-muls AFTER tanh1/h1: layer 1's tanh feeds
                # the long recurrence chain, layer 2's only next phase's
                # sig2.
                if hasc2:
                    tanh_emit(c12, tt12, [1])
                    hoff = (tc2 % RS) * BS
                    hmul(h2_hist[:, hoff:hoff + BS], st["s2"], tt12, 1)
                # x-part MMs for step p+1, one phase ahead (see above);
                # emitted BEFORE g2's h-part so the PE reaches them while
                # the h-part still waits on this phase's h2-muls.
                if p + 1 < nsteps:
                    if (p + 1) % XCH == 0:
                        x_next = load_x(p + 1)
                    g1n = gp.tile([HID, 2 * NGc], F32, tag="g")
                    emit_x_mms(g1n, p + 1, x_next, last=False)
                    st["g1n"] = g1n

                if hasg2 and t2 > 0:
                    for ch in (0, 1):
                        poff = ((t2 - 1) % RS) * BS + CB * ch
                        for m in range(4):
                            sl = slice(NGc * ch + CB * m,
                                       NGc * ch + CB * (m + 1))
                            nc.tensor.matmul(
                                g2[:, sl], whh1[:, HID * m:HID * (m + 1)],
                                h2_hist[:, poff:poff + CB],
                                start=False, stop=(m == 3))

                if has1:
                    st["h1"] = h1
                if hasg2:
                    st["s2"] = sig(g2, "2", SMODE in ('m2', 'mm'))
                st["c12"] = c12

                # ---- output projection + pack for steps (tc2-1, tc2),
                # overlapped with the recurrence.  Four [14,512] projection
                # chunks share one psum bank at quadrant partition bases
                # (0/32/64/96), so ONE [110,512] staging copy (DVE cost
                # depends on free size only) replaces four copies; the pack
                # DMAs then read the quadrant stripes.  (GPSIMD cannot
                # read PSUM, so the copy runs on the vector engine.)
                if hasc2 and tc2 % 2 == 1:
                    c = tc2 // 2
                    q = c % 3
                    if q == 0:
                        proj_pp = projp.tile([2 * 32 + 2 * TGT, 512], F32,
                                             tag="pp")
                        proj_pend = []
                    rcol = ((2 * c) % RS) * BS
                    nc.tensor.matmul(proj_pp[32 * q:32 * q + 2 * TGT, :],
                                     wms[:],
                                     h2_hist[:, rcol:rcol + 512],
                                     start=True, stop=True)
                    proj_pend.append((q, c))
                    if q == 2 or c == nproj - 1:
                        stt = stagep.tile([2 * 32 + 2 * TGT, 512], F32,
                                          tag="st")
                        nc.vector.tensor_copy(stt[:], proj_pp[:])
                        for qq, cc_ in proj_pend:
                            g = cc_ % PGRP
                            cc = cc_ // PGRP
                            dst = slice(512 * cc, 512 * (cc + 1))
                            base = 32 * qq
                            nc.sync.dma_start(
                                out=means_sb[TGT * g:TGT * (g + 1), dst],
                                in_=stt[base:base + TGT, :])
                            nc.sync.dma_start(
                                out=sigraw_sb[TGT * g:TGT * (g + 1), dst],
                                in_=stt[base + TGT:base + 2 * TGT, :])
            ctx.close()

            if True:
                # ---- epilogue: softplus + denorm ----
                nf = 2 * pcc  # broadcast factor along free dim
                std_bc = stdp[:, :].unsqueeze(1).broadcast_to(
                    [TGT * PGRP, nf, BS])
                mean_bc = meanp[:, :].unsqueeze(1).broadcast_to(
                    [TGT * PGRP, nf, BS])
                # softplus(x+bs) = ln(1 + exp(x+bs)); Softplus itself has no
                # ACT table set, but exp and ln share one.
                sigsp = singles.tile([TGT * PGRP, 512 * pcc], F32, tag="sigsp")
                nc.scalar.activation(sigsp[:], sigraw_sb[:], AF.Exp,
                                     bias=bsp[:, :])
                nc.scalar.activation(sigsp[:], sigsp[:], AF.Ln, bias=1.0)
                nc.vector.tensor_mul(sigsp[:], sigsp[:], std_bc)
                nc.vector.tensor_mul(means_sb[:], means_sb[:], std_bc)
                nc.vector.tensor_add(means_sb[:], means_sb[:], mean_bc)
                nc.sync.dma_start(out=means_d, in_=means_sb[:])
                nc.sync.dma_start(out=sigmas_d, in_=sigsp[:])

    nc.finalize()
    return nc


# --------------------------------------------------------------------------
# top-level entry
# --------------------------------------------------------------------------

def _pack_norm(arr):
    """[BS,7] per-core norm stats -> [112, BS] tiled PGRP times."""
    a = arr.T.astype(np.float32)                 # [7, BS]
    return np.tile(a, (PGRP, 1)).astype(np.float32)


def run(inputs, trace=False, nsteps=W):
    inputs = {k: np.asarray(v) for k, v in inputs.items()}
    inp, nmean, nstd = _host_prep(inputs)
    wts = _host_weights(inputs)
    bf = ml_dtypes.bfloat16

    bm = inputs["bm"].astype(np.float32)
    bs_ = inputs["bs"].astype(np.float32)

    in_maps = []
    for k in range(NCORES):
        bsl = slice(k * BS, (k + 1) * BS)
        # [BS, nsteps, 66] -> [66, nsteps*BS] with col = t*BS + b
        xi = np.concatenate(
            [inp[bsl, :nsteps, :], np.ones((BS, nsteps, 1), np.float32)],
            axis=-1)
        xiT = np.ascontiguousarray(xi.transpose(2, 1, 0).reshape(INPX, -1))
        std_c = nstd[bsl]                        # [BS, 7]
        mean_c = nmean[bsl]
        m = dict(wts)
        m["inp"] = xiT.astype(bf)
        m["stdp"] = _pack_norm(std_c)
        # fold bm*std + mean into the additive term
        m["meanp"] = _pack_norm(bm[None, :] * std_c + mean_c)
        m["bsp"] = np.tile(bs_, PGRP)[:, None].astype(np.float32)
        in_maps.append(m)

    key = nsteps
    if key not in _CACHE:
        _CACHE[key] = build_module(nsteps)
    nc = _CACHE[key]

    res = bass_utils.run_bass_kernel_spmd(
        nc, in_maps, core_ids=list(range(NCORES)), trace=False)

    nproj = nsteps * BS // 512
    pcc = max(1, nproj // PGRP)
    out = np.empty((B, nsteps, 2 * TGT), np.float32)
    for k in range(NCORES):
        r = res.results[k]
        for name, off in (("means", 0), ("sigmas", TGT)):
            a = r[name].reshape(PGRP, TGT, pcc, 2, BS)
            # [g, o, cc, tau, b] -> [b, cc, g, tau, o]
            a = a.transpose(4, 2, 0, 3, 1).reshape(BS, nsteps, TGT)
            out[k * BS:(k + 1) * BS, :, off:off + TGT] = a
    return out, res.exec_time_ns


def kernel(**inputs):
    out, _ = run(inputs, trace=False)
    return out

